# revision 15
# baseline (speedup 1.0000x reference)
"""Trainium2 Bass kernel for AggregationRebuild (GNN message passing).

Reference computation (N=2048, K=8, L=64, D=64):
    sub_sim[n,k]  = sim[n, index[n,k]] / 0.5
    W             = softmax(sub_sim, axis=-1)                 # [N, K]
    out[n]        = sum_k W[n,k] * emb[index[n,k]]            # [N, L*D]

Sharding: rows split across 8 cores (256 rows each); emb replicated.

Per-core program (2 row-chunks of 128 rows = partition dim):
  - dma_gather 256B blocks of the core's sim slab containing each
    sub-similarity element; extract with an iota==rmod one-hot
    (tensor_scalar is_equal*2 folds the /T), multiply + free-axis reduce.
  - softmax over K on ACT (Exp with per-partition -max bias, fused sum).
  - dma_gather each neighbor's 16KB feature row into its own partition;
    weighted sum: ACT multiplies by the per-partition softmax weight
    (activation Copy with scale AP), DVE accumulates. All fp32.
"""

import sys
import types

import numpy as np

import concourse.bass as bass
import concourse.tile as tile
from concourse import bacc, library_config, mybir
from concourse.bass_utils import run_bass_kernel_spmd

F32 = mybir.dt.float32
I16 = mybir.dt.int16

N, K, L, D = 2048, 8, 64, 64
LD = L * D                      # 4096
NCORES = 8
RPD = N // NCORES               # 256 rows per device
NCH = RPD // 128                # 2 chunks of 128 rows
SBLK = 64                       # sim gather block (elements) = 256B
NBLK = N // SBLK                # 32 blocks per sim row
F_BUFS = 6
O_BUFS = 2

_cache: dict = {}


def _install_axon_ntff_hook():
    """The agent image's antenv lacks axon_hooks; synthesize it so
    run_bass_kernel_spmd(trace=True) can capture NTFF profiles."""
    try:
        from antenv.axon_hooks import get_axon_ntff_profile_hook  # noqa: F401
        return
    except ImportError:
        pass
    try:
        from trn_agent_boot.trn_boot import _ntff_profile_via_ctypes
        hook = _ntff_profile_via_ctypes("/opt/axon/libaxon_pjrt.so")
    except Exception:
        hook = None
    mod = types.ModuleType("antenv.axon_hooks")
    mod.get_axon_ntff_profile_hook = lambda: hook
    mod.set_axon_ntff_profile_hook = lambda h: None
    sys.modules["antenv.axon_hooks"] = mod


def build():
    nc = bacc.Bacc("TRN2", target_bir_lowering=False, debug=False,
                   num_devices=NCORES)

    emb = nc.dram_tensor("emb", [N, LD], F32, kind="ExternalInput")
    simb = nc.dram_tensor("simb", [RPD * NBLK, SBLK], F32, kind="ExternalInput")
    # per-k sim tables (16 wrapped cols each, both chunks) then per-(c,k)
    # feature tables (8 wrapped cols each)
    gidx = nc.dram_tensor("gidx", [128, K * 16 + NCH * K * 8], I16,
                          kind="ExternalInput")
    rmod = nc.dram_tensor("rmod", [128, NCH * K], F32, kind="ExternalInput")
    iotac = nc.dram_tensor("iotac", [128, SBLK], F32, kind="ExternalInput")
    wout = nc.dram_tensor("wout", [RPD, K], F32, kind="ExternalOutput")
    eout = nc.dram_tensor("eout", [RPD, LD], F32, kind="ExternalOutput")

    with tile.TileContext(nc) as tc:
        with (
            tc.tile_pool(name="const", bufs=1) as cpool,
            tc.tile_pool(name="bblk", bufs=K) as bpool,
            tc.tile_pool(name="hh", bufs=2) as hpool,
            tc.tile_pool(name="sm", bufs=2) as smpool,
            tc.tile_pool(name="stat", bufs=8) as stpool,
            tc.tile_pool(name="feat", bufs=F_BUFS) as fpool,
            tc.tile_pool(name="acc", bufs=O_BUFS) as apool,
        ):
            gidx_t = cpool.tile([128, K * 16 + NCH * K * 8], I16)
            nc.sync.dma_start(gidx_t[:], gidx[:])
            rmod_t = cpool.tile([128, NCH * K], F32)
            nc.sync.dma_start(rmod_t[:], rmod[:])
            iota_t = cpool.tile([128, SBLK], F32)
            nc.sync.dma_start(iota_t[:], iotac[:])

            nc.gpsimd.load_library(library_config.mlp)

            # gathers: chunk-0 features interleaved with per-k sim blocks
            # (each sim gather covers both chunks: 256 idxs), then chunk-1
            # features.
            bts = []
            fts = {}
            for k in range(K):
                ci = K * 16 + (0 * K + k) * 8
                ft = fpool.tile([128, 1, LD], F32, tag="ft")
                nc.gpsimd.dma_gather(
                    ft[:], emb[:], gidx_t[:, ci:ci + 8], 128, 128, LD)
                fts[(0, k)] = ft
                bt = bpool.tile([128, NCH, SBLK], F32)
                nc.gpsimd.dma_gather(
                    bt[:], simb[:], gidx_t[:, k * 16:(k + 1) * 16],
                    NCH * 128, NCH * 128, SBLK)
                bts.append(bt)
            for k in range(K):
                ci = K * 16 + (1 * K + k) * 8
                ft = fpool.tile([128, 1, LD], F32, tag="ft")
                nc.gpsimd.dma_gather(
                    ft[:], emb[:], gidx_t[:, ci:ci + 8], 128, 128, LD)
                fts[(1, k)] = ft

            # --- extraction + softmax for both chunks (small DVE/ACT ops) ---
            ws = []
            for c in range(NCH):
                sub = smpool.tile([128, K], F32, tag="sub")
                for k in range(K):
                    j = c * K + k
                    h = hpool.tile([128, SBLK], F32, tag="h")
                    # h = (iota == rmod) * 2   (folds the 1/T scale)
                    nc.vector.tensor_scalar(
                        out=h[:], in0=iota_t[:], scalar1=rmod_t[:, j:j + 1],
                        scalar2=2.0, op0=mybir.AluOpType.is_equal,
                        op1=mybir.AluOpType.mult)
                    p = hpool.tile([128, SBLK], F32, tag="p")
                    nc.vector.tensor_tensor(
                        out=p[:], in0=h[:], in1=bts[k][:, c, :],
                        op=mybir.AluOpType.mult)
                    nc.vector.reduce_sum(
                        out=sub[:, k:k + 1], in_=p[:],
                        axis=mybir.AxisListType.X)

                negmx = stpool.tile([128, 1], F32, tag="negmx")
                nc.vector.reduce_max(out=negmx[:], in_=sub[:],
                                     axis=mybir.AxisListType.X, negate=True)
                e = smpool.tile([128, K], F32, tag="e")
                ssum = stpool.tile([128, 1], F32, tag="ssum")
                nc.scalar.activation(
                    out=e[:], in_=sub[:],
                    func=mybir.ActivationFunctionType.Exp,
                    bias=negmx[:], scale=1.0, accum_out=ssum[:])
                rcp = stpool.tile([128, 1], F32, tag="rcp")
                nc.vector.reciprocal(rcp[:], ssum[:])
                w = smpool.tile([128, K], F32, tag="w")
                nc.vector.tensor_scalar(
                    out=w[:], in0=e[:], scalar1=rcp[:, 0:1], scalar2=None,
                    op0=mybir.AluOpType.mult)
                nc.sync.dma_start(wout[c * 128:(c + 1) * 128, :], w[:])
                ws.append(w)

            # --- weighted sum: in-place ACT scaling + DVE pairwise adds ---
            for c in range(NCH):
                w = ws[c]

                def scale(k):
                    f = fts[(c, k)][:, 0, :]
                    nc.scalar.activation(
                        out=f, in_=f,
                        func=mybir.ActivationFunctionType.Copy,
                        scale=w[:, k:k + 1])

                def add(ka, kb):
                    fa = fts[(c, ka)][:, 0, :]
                    fb = fts[(c, kb)][:, 0, :]
                    nc.vector.tensor_tensor(out=fa, in0=fa, in1=fb,
                                            op=mybir.AluOpType.add)

                for k in range(K):
                    scale(k)
                    if k % 2 == 1:
                        add(k - 1, k)      # ft[k-1] += ft[k]
                add(0, 2)
                add(4, 6)
                acc = apool.tile([128, LD], F32)
                nc.vector.tensor_tensor(
                    out=acc[:], in0=fts[(c, 0)][:, 0, :],
                    in1=fts[(c, 4)][:, 0, :], op=mybir.AluOpType.add)
                nc.sync.dma_start(eout[c * 128:(c + 1) * 128, :], acc[:])

    nc.compile()
    return nc


def prep_inputs(similarity_matrix, batch_emb_om, index):
    """Shard + marshal full inputs into per-core input maps."""
    sim = np.ascontiguousarray(similarity_matrix, dtype=np.float32)
    emb = np.ascontiguousarray(
        np.asarray(batch_emb_om, dtype=np.float32).reshape(N, LD))
    idx = np.asarray(index, dtype=np.int64).astype(np.int32)

    iotac = np.tile(np.arange(SBLK, dtype=np.float32), (128, 1))

    def wrap(lst):
        # dma_gather order: item i read from idxs[i%16, i//16]; replicate to
        # all 8 GPSIMD core groups (16 partitions each).
        return np.tile(lst.astype(np.int16).reshape(-1, 16).T, (8, 1))

    in_maps = []
    for d in range(NCORES):
        idx_d = idx[d * RPD:(d + 1) * RPD]            # [256, K]
        gidx_d = np.zeros((128, K * 16 + NCH * K * 8), dtype=np.int16)
        rmod_d = np.zeros((128, NCH * K), dtype=np.float32)
        rows_all = np.arange(RPD)
        for k in range(K):
            cols = idx_d[rows_all, k]
            sim_lst = (rows_all * NBLK + (cols >> 6))  # [256]
            gidx_d[:, k * 16:(k + 1) * 16] = wrap(sim_lst)
        for c in range(NCH):
            rows = np.arange(c * 128, (c + 1) * 128)
            for k in range(K):
                cols = idx_d[rows, k]
                ci = K * 16 + (c * K + k) * 8
                gidx_d[:, ci:ci + 8] = wrap(cols)
                rmod_d[:, c * K + k] = (cols & 63).astype(np.float32)
        in_maps.append({
            "emb": emb,
            "simb": sim[d * RPD:(d + 1) * RPD].reshape(RPD * NBLK, SBLK),
            "gidx": gidx_d,
            "rmod": rmod_d,
            "iotac": iotac,
        })
    return in_maps


def assemble_outputs(results):
    wfull = np.concatenate([r["wout"] for r in results], axis=0)
    efull = np.concatenate([r["eout"] for r in results], axis=0)
    return wfull.reshape(N, K), efull.reshape(N, L, D)


def kernel(similarity_matrix, batch_emb_om, index, trace=False):
    if trace:
        _install_axon_ntff_hook()
    if "nc" not in _cache:
        _cache["nc"] = build()
    nc = _cache["nc"]
    in_maps = prep_inputs(similarity_matrix, batch_emb_om, index)
    res = run_bass_kernel_spmd(nc, in_maps, core_ids=list(range(NCORES)),
                               trace=trace)
    out = assemble_outputs(res.results)
    if trace:
        _cache["last_result"] = res
    return out


# revision 17
# speedup vs baseline: 1.1347x; 1.1347x over previous
"""Trainium2 Bass kernel for AggregationRebuild (GNN message passing).

Reference computation (N=2048, K=8, L=64, D=64):
    sub_sim[n,k]  = sim[n, index[n,k]] / 0.5
    W             = softmax(sub_sim, axis=-1)                 # [N, K]
    out[n]        = sum_k W[n,k] * emb[index[n,k]]            # [N, L*D]

Sharding: rows split across 8 cores (256 rows each); emb replicated.

Per-core program (2 row-chunks of 128 rows = partition dim):
  - dma_gather 256B blocks of the core's sim slab containing each
    sub-similarity element; extract with an iota==rmod one-hot
    (tensor_scalar is_equal*2 folds the /T), multiply + free-axis reduce.
  - softmax over K on ACT (Exp with per-partition -max bias, fused sum).
  - dma_gather each neighbor's 16KB feature row into its own partition;
    weighted sum: ACT multiplies by the per-partition softmax weight
    (activation Copy with scale AP), DVE accumulates. All fp32.
"""

import sys
import types

import numpy as np

import concourse.bass as bass
import concourse.tile as tile
from concourse import bacc, library_config, mybir
from concourse.bass_utils import run_bass_kernel_spmd

F32 = mybir.dt.float32
I16 = mybir.dt.int16

N, K, L, D = 2048, 8, 64, 64
LD = L * D                      # 4096
NCORES = 8
RPD = N // NCORES               # 256 rows per device
NCH = RPD // 128                # 2 chunks of 128 rows
SBLK = 64                       # sim gather block (elements) = 256B
NBLK = N // SBLK                # 32 blocks per sim row
F_BUFS = 6
O_BUFS = 2

_cache: dict = {}


def _install_axon_ntff_hook():
    """The agent image's antenv lacks axon_hooks; synthesize it so
    run_bass_kernel_spmd(trace=True) can capture NTFF profiles."""
    try:
        from antenv.axon_hooks import get_axon_ntff_profile_hook  # noqa: F401
        return
    except ImportError:
        pass
    try:
        from trn_agent_boot.trn_boot import _ntff_profile_via_ctypes
        hook = _ntff_profile_via_ctypes("/opt/axon/libaxon_pjrt.so")
    except Exception:
        hook = None
    mod = types.ModuleType("antenv.axon_hooks")
    mod.get_axon_ntff_profile_hook = lambda: hook
    mod.set_axon_ntff_profile_hook = lambda h: None
    sys.modules["antenv.axon_hooks"] = mod


def build():
    nc = bacc.Bacc("TRN2", target_bir_lowering=False, debug=False,
                   num_devices=NCORES)

    emb = nc.dram_tensor("emb", [N, LD], F32, kind="ExternalInput")
    simb = nc.dram_tensor("simb", [RPD * NBLK, SBLK], F32, kind="ExternalInput")
    # per-k sim tables (16 wrapped cols each, both chunks) then per-(c,k)
    # feature tables (8 wrapped cols each)
    gidx = nc.dram_tensor("gidx", [128, K * 16 + NCH * K * 8], I16,
                          kind="ExternalInput")
    rmod = nc.dram_tensor("rmod", [128, NCH * K], F32, kind="ExternalInput")
    iotac = nc.dram_tensor("iotac", [128, SBLK], F32, kind="ExternalInput")
    wout = nc.dram_tensor("wout", [RPD, K], F32, kind="ExternalOutput")
    eout = nc.dram_tensor("eout", [RPD, LD], F32, kind="ExternalOutput")

    with tile.TileContext(nc) as tc:
        with (
            tc.tile_pool(name="const", bufs=1) as cpool,
            tc.tile_pool(name="bblk", bufs=K) as bpool,
            tc.tile_pool(name="hh", bufs=2) as hpool,
            tc.tile_pool(name="sm", bufs=2) as smpool,
            tc.tile_pool(name="stat", bufs=8) as stpool,
            tc.tile_pool(name="feat", bufs=F_BUFS) as fpool,
            tc.tile_pool(name="acc", bufs=O_BUFS) as apool,
        ):
            gidx_t = cpool.tile([128, K * 16 + NCH * K * 8], I16)
            nc.sync.dma_start(gidx_t[:], gidx[:])
            rmod_t = cpool.tile([128, NCH * K], F32)
            nc.sync.dma_start(rmod_t[:], rmod[:])
            iota_t = cpool.tile([128, SBLK], F32)
            nc.sync.dma_start(iota_t[:], iotac[:])

            nc.gpsimd.load_library(library_config.mlp)

            # gathers: sim-block gathers (tiny, gate the softmax) densely
            # interleaved among the first feature gathers so the weights are
            # ready by the time feature consumption must start.
            bts = [None] * K
            fts = {}

            def feat_gather(c, k):
                ci = K * 16 + (c * K + k) * 8
                ft = fpool.tile([128, 1, LD], F32, tag="ft")
                nc.gpsimd.dma_gather(
                    ft[:], emb[:], gidx_t[:, ci:ci + 8], 128, 128, LD)
                fts[(c, k)] = ft

            def sim_gather(k):
                bt = bpool.tile([128, NCH, SBLK], F32)
                nc.gpsimd.dma_gather(
                    bt[:], simb[:], gidx_t[:, k * 16:(k + 1) * 16],
                    NCH * 128, NCH * 128, SBLK)
                bts[k] = bt

            feat_gather(0, 0)
            sim_gather(0)
            sim_gather(1)
            feat_gather(0, 1)
            sim_gather(2)
            sim_gather(3)
            feat_gather(0, 2)
            sim_gather(4)
            sim_gather(5)
            feat_gather(0, 3)
            sim_gather(6)
            sim_gather(7)
            for k in range(4, K):
                feat_gather(0, k)
            for k in range(K):
                feat_gather(1, k)

            # --- extraction + softmax for both chunks (small DVE/ACT ops) ---
            ws = []
            for c in range(NCH):
                sub = smpool.tile([128, K], F32, tag="sub")
                for k in range(K):
                    j = c * K + k
                    h = hpool.tile([128, SBLK], F32, tag="h")
                    # h = (iota == rmod) * 2   (folds the 1/T scale)
                    nc.vector.tensor_scalar(
                        out=h[:], in0=iota_t[:], scalar1=rmod_t[:, j:j + 1],
                        scalar2=2.0, op0=mybir.AluOpType.is_equal,
                        op1=mybir.AluOpType.mult)
                    p = hpool.tile([128, SBLK], F32, tag="p")
                    nc.vector.tensor_tensor(
                        out=p[:], in0=h[:], in1=bts[k][:, c, :],
                        op=mybir.AluOpType.mult)
                    nc.vector.reduce_sum(
                        out=sub[:, k:k + 1], in_=p[:],
                        axis=mybir.AxisListType.X)

                negmx = stpool.tile([128, 1], F32, tag="negmx")
                nc.vector.reduce_max(out=negmx[:], in_=sub[:],
                                     axis=mybir.AxisListType.X, negate=True)
                e = smpool.tile([128, K], F32, tag="e")
                ssum = stpool.tile([128, 1], F32, tag="ssum")
                nc.scalar.activation(
                    out=e[:], in_=sub[:],
                    func=mybir.ActivationFunctionType.Exp,
                    bias=negmx[:], scale=1.0, accum_out=ssum[:])
                rcp = stpool.tile([128, 1], F32, tag="rcp")
                nc.vector.reciprocal(rcp[:], ssum[:])
                w = smpool.tile([128, K], F32, tag="w")
                nc.vector.tensor_scalar(
                    out=w[:], in0=e[:], scalar1=rcp[:, 0:1], scalar2=None,
                    op0=mybir.AluOpType.mult)
                nc.sync.dma_start(wout[c * 128:(c + 1) * 128, :], w[:])
                ws.append(w)

            # --- weighted sum: chain accumulation, freeing F slots in
            # gather order (acc = w0*f0; then in-place scale + add per k) ---
            for c in range(NCH):
                w = ws[c]
                acc = apool.tile([128, LD], F32)
                nc.scalar.activation(
                    out=acc[:], in_=fts[(c, 0)][:, 0, :],
                    func=mybir.ActivationFunctionType.Copy,
                    scale=w[:, 0:1])
                for k in range(1, K):
                    f = fts[(c, k)][:, 0, :]
                    nc.scalar.activation(
                        out=f, in_=f,
                        func=mybir.ActivationFunctionType.Copy,
                        scale=w[:, k:k + 1])
                    nc.vector.tensor_tensor(out=acc[:], in0=acc[:], in1=f,
                                            op=mybir.AluOpType.add)
                nc.sync.dma_start(eout[c * 128:(c + 1) * 128, :], acc[:])

    nc.compile()
    return nc


def prep_inputs(similarity_matrix, batch_emb_om, index):
    """Shard + marshal full inputs into per-core input maps."""
    sim = np.ascontiguousarray(similarity_matrix, dtype=np.float32)
    emb = np.ascontiguousarray(
        np.asarray(batch_emb_om, dtype=np.float32).reshape(N, LD))
    idx = np.asarray(index, dtype=np.int64).astype(np.int32)

    iotac = np.tile(np.arange(SBLK, dtype=np.float32), (128, 1))

    def wrap(lst):
        # dma_gather order: item i read from idxs[i%16, i//16]; replicate to
        # all 8 GPSIMD core groups (16 partitions each).
        return np.tile(lst.astype(np.int16).reshape(-1, 16).T, (8, 1))

    in_maps = []
    for d in range(NCORES):
        idx_d = idx[d * RPD:(d + 1) * RPD]            # [256, K]
        gidx_d = np.zeros((128, K * 16 + NCH * K * 8), dtype=np.int16)
        rmod_d = np.zeros((128, NCH * K), dtype=np.float32)
        rows_all = np.arange(RPD)
        for k in range(K):
            cols = idx_d[rows_all, k]
            sim_lst = (rows_all * NBLK + (cols >> 6))  # [256]
            gidx_d[:, k * 16:(k + 1) * 16] = wrap(sim_lst)
        for c in range(NCH):
            rows = np.arange(c * 128, (c + 1) * 128)
            for k in range(K):
                cols = idx_d[rows, k]
                ci = K * 16 + (c * K + k) * 8
                gidx_d[:, ci:ci + 8] = wrap(cols)
                rmod_d[:, c * K + k] = (cols & 63).astype(np.float32)
        in_maps.append({
            "emb": emb,
            "simb": sim[d * RPD:(d + 1) * RPD].reshape(RPD * NBLK, SBLK),
            "gidx": gidx_d,
            "rmod": rmod_d,
            "iotac": iotac,
        })
    return in_maps


def assemble_outputs(results):
    wfull = np.concatenate([r["wout"] for r in results], axis=0)
    efull = np.concatenate([r["eout"] for r in results], axis=0)
    return wfull.reshape(N, K), efull.reshape(N, L, D)


def kernel(similarity_matrix, batch_emb_om, index, trace=False):
    if trace:
        _install_axon_ntff_hook()
    if "nc" not in _cache:
        _cache["nc"] = build()
    nc = _cache["nc"]
    in_maps = prep_inputs(similarity_matrix, batch_emb_om, index)
    res = run_bass_kernel_spmd(nc, in_maps, core_ids=list(range(NCORES)),
                               trace=trace)
    out = assemble_outputs(res.results)
    if trace:
        _cache["last_result"] = res
    return out


# revision 45
# speedup vs baseline: 1.1722x; 1.0331x over previous
"""Trainium2 Bass kernel for AggregationRebuild (GNN message passing).

Reference computation (N=2048, K=8, L=64, D=64):
    sub_sim[n,k]  = sim[n, index[n,k]] / 0.5
    W             = softmax(sub_sim, axis=-1)                 # [N, K]
    out[n]        = sum_k W[n,k] * emb[index[n,k]]            # [N, L*D]

Sharding: rows split across 8 cores (256 rows each); emb replicated.

Per-core program (2 row-chunks of 128 rows = partition dim):
  - dma_gather 256B blocks of the core's sim slab containing each
    sub-similarity element; extract with an iota==rmod one-hot
    (tensor_scalar is_equal*2 folds the /T), multiply + free-axis reduce.
  - softmax over K on ACT (Exp with per-partition -max bias, fused sum).
  - dma_gather each neighbor's 16KB feature row into its own partition
    (SWDGE ucode path; ring doubled so 7 gathers pipeline), production
    alternating between chunks so both weighted-sum consumers run
    concurrently:
      chunk 0: ACT scales in place (Copy with per-partition scale AP),
               DVE chain-adds, split tail + pipelined eout DMA;
      chunk 1: PE diagonal matmuls (psum += diag(w_k) @ F_k) accumulating
               in PSUM — fp32 matmul is bit-accurate here since every
               output has exactly one nonzero product.
  All fp32; measured rel err ~1.2e-6 end to end.
"""

import sys
import types

import numpy as np

import concourse.bass as bass
import concourse.tile as tile
from concourse import bacc, library_config, mybir
from concourse.bass_utils import run_bass_kernel_spmd

F32 = mybir.dt.float32
I16 = mybir.dt.int16

N, K, L, D = 2048, 8, 64, 64
LD = L * D                      # 4096
NCORES = 8
RPD = N // NCORES               # 256 rows per device
NCH = RPD // 128                # 2 chunks of 128 rows
SBLK = 64                       # sim gather block (elements) = 256B
NBLK = N // SBLK                # 32 blocks per sim row
# 7 F-buffers needs the doubled SWDGE descriptor ring (DMA_SCRATCH below);
# with the default 16KB ring, 7 in-flight gathers corrupt results on HW.
F_BUFS = 7
O_BUFS = 2
DMA_SCRATCH = 32768
MM_N = 512                      # matmul free-dim (one PSUM bank)

_cache: dict = {}


def _install_axon_ntff_hook():
    """The agent image's antenv lacks axon_hooks; synthesize it so
    run_bass_kernel_spmd(trace=True) can capture NTFF profiles."""
    try:
        from antenv.axon_hooks import get_axon_ntff_profile_hook  # noqa: F401
        return
    except ImportError:
        pass
    try:
        from trn_agent_boot.trn_boot import _ntff_profile_via_ctypes
        hook = _ntff_profile_via_ctypes("/opt/axon/libaxon_pjrt.so")
    except Exception:
        hook = None
    mod = types.ModuleType("antenv.axon_hooks")
    mod.get_axon_ntff_profile_hook = lambda: hook
    mod.set_axon_ntff_profile_hook = lambda h: None
    sys.modules["antenv.axon_hooks"] = mod


def build():
    nc = bacc.Bacc("TRN2", target_bir_lowering=False, debug=False,
                   num_devices=NCORES,
                   dynamic_dma_scratch_size=DMA_SCRATCH)

    emb = nc.dram_tensor("emb", [N, LD], F32, kind="ExternalInput")
    simb = nc.dram_tensor("simb", [RPD * NBLK, SBLK], F32, kind="ExternalInput")
    # per-k sim tables (16 wrapped cols each, both chunks) then per-(c,k)
    # feature tables (8 wrapped cols each)
    gidx = nc.dram_tensor("gidx", [128, K * 16 + NCH * K * 8], I16,
                          kind="ExternalInput")
    # merged f32 constants: rmod [16] | iota [64] | identity [128]
    cst = nc.dram_tensor("cst", [128, NCH * K + SBLK + 128], F32,
                         kind="ExternalInput")
    wout = nc.dram_tensor("wout", [RPD, K], F32, kind="ExternalOutput")
    eout = nc.dram_tensor("eout", [RPD, LD], F32, kind="ExternalOutput")

    with tile.TileContext(nc) as tc:
        with (
            tc.tile_pool(name="const", bufs=1) as cpool,
            tc.tile_pool(name="bblk", bufs=K) as bpool,
            tc.tile_pool(name="hh", bufs=2) as hpool,
            tc.tile_pool(name="sm", bufs=2) as smpool,
            tc.tile_pool(name="stat", bufs=8) as stpool,
            tc.tile_pool(name="feat", bufs=F_BUFS) as fpool,
            tc.tile_pool(name="acc", bufs=O_BUFS) as apool,
            tc.tile_pool(name="wd", bufs=K) as dpool,
            tc.tile_pool(name="ps", bufs=1, space="PSUM") as pspool,
        ):
            gidx_t = cpool.tile([128, K * 16 + NCH * K * 8], I16)
            nc.sync.dma_start(gidx_t[:], gidx[:])
            cst_t = cpool.tile([128, NCH * K + SBLK + 128], F32)
            nc.sync.dma_start(cst_t[:], cst[:])
            rmod_t = cst_t[:, 0:NCH * K]
            iota_t = cst_t[:, NCH * K:NCH * K + SBLK]
            ident_t = cst_t[:, NCH * K + SBLK:]

            nc.gpsimd.load_library(library_config.mlp)

            # gathers: sim-block gathers (tiny, gate the softmax) densely
            # interleaved among the first feature gathers so the weights are
            # ready by the time feature consumption must start.
            bts = [None] * K
            fts = {}

            def feat_gather(c, k):
                ci = K * 16 + (c * K + k) * 8
                ft = fpool.tile([128, 1, LD], F32, tag="ft")
                nc.gpsimd.dma_gather(
                    ft[:], emb[:], gidx_t[:, ci:ci + 8], 128, 128, LD)
                fts[(c, k)] = ft

            def sim_gather(k):
                bt = bpool.tile([128, NCH, SBLK], F32)
                nc.gpsimd.dma_gather(
                    bt[:], simb[:], gidx_t[:, k * 16:(k + 1) * 16],
                    NCH * 128, NCH * 128, SBLK)
                bts[k] = bt

            def bt_slice(c, k):
                return bts[k][:, c, :]

            # Alternate chunks so both consumers (chunk0: ACT/DVE,
            # chunk1: PE) get work continuously; sim gathers early so the
            # softmax unblocks consumption as soon as possible.
            feat_gather(0, 0)
            feat_gather(1, 0)
            sim_gather(0)
            sim_gather(1)
            feat_gather(0, 1)
            sim_gather(2)
            sim_gather(3)
            feat_gather(1, 1)
            sim_gather(4)
            sim_gather(5)
            feat_gather(0, 2)
            sim_gather(6)
            sim_gather(7)
            feat_gather(1, 2)
            for k in range(3, K - 1):
                feat_gather(0, k)
                feat_gather(1, k)
            # last pair swapped: PE's final tile lands one slot earlier,
            # chunk-0 (fast split tail) is the absolute last
            feat_gather(1, K - 1)
            feat_gather(0, K - 1)

            # --- extraction + softmax for both chunks (small DVE/ACT ops) ---
            ws = []
            for c in range(NCH):
                sub = smpool.tile([128, K], F32, tag="sub")
                for k in range(K):
                    j = c * K + k
                    h = hpool.tile([128, SBLK], F32, tag="h")
                    # h = (iota == rmod) * 2   (folds the 1/T scale)
                    nc.vector.tensor_scalar(
                        out=h[:], in0=iota_t[:], scalar1=rmod_t[:, j:j + 1],
                        scalar2=2.0, op0=mybir.AluOpType.is_equal,
                        op1=mybir.AluOpType.mult)
                    p = hpool.tile([128, SBLK], F32, tag="p")
                    nc.vector.tensor_tensor(
                        out=p[:], in0=h[:], in1=bt_slice(c, k),
                        op=mybir.AluOpType.mult)
                    nc.vector.reduce_sum(
                        out=sub[:, k:k + 1], in_=p[:],
                        axis=mybir.AxisListType.X)

                negmx = stpool.tile([128, 1], F32, tag="negmx")
                nc.vector.reduce_max(out=negmx[:], in_=sub[:],
                                     axis=mybir.AxisListType.X, negate=True)
                e = smpool.tile([128, K], F32, tag="e")
                ssum = stpool.tile([128, 1], F32, tag="ssum")
                nc.scalar.activation(
                    out=e[:], in_=sub[:],
                    func=mybir.ActivationFunctionType.Exp,
                    bias=negmx[:], scale=1.0, accum_out=ssum[:])
                rcp = stpool.tile([128, 1], F32, tag="rcp")
                nc.vector.reciprocal(rcp[:], ssum[:])
                w = smpool.tile([128, K], F32, tag="w")
                nc.vector.tensor_scalar(
                    out=w[:], in0=e[:], scalar1=rcp[:, 0:1], scalar2=None,
                    op0=mybir.AluOpType.mult)
                nc.sync.dma_start(wout[c * 128:(c + 1) * 128, :], w[:])
                ws.append(w)

            # diag weight matrices for the chunk-1 PE path
            wds = []
            for k in range(K):
                wd = dpool.tile([128, 128], F32)
                nc.vector.tensor_scalar(
                    out=wd[:], in0=ident_t[:], scalar1=ws[1][:, k:k + 1],
                    scalar2=None, op0=mybir.AluOpType.mult)
                wds.append(wd)

            # --- chunk 0 weighted sum: ACT scaling + DVE chain adds ---
            w = ws[0]
            acc = apool.tile([128, LD], F32)
            nc.scalar.activation(
                out=acc[:], in_=fts[(0, 0)][:, 0, :],
                func=mybir.ActivationFunctionType.Copy, scale=w[:, 0:1])
            for k in range(1, K - 1):
                f = fts[(0, k)][:, 0, :]
                nc.scalar.activation(
                    out=f, in_=f, func=mybir.ActivationFunctionType.Copy,
                    scale=w[:, k:k + 1])
                nc.vector.tensor_tensor(out=acc[:], in0=acc[:], in1=f,
                                        op=mybir.AluOpType.add)
            # last k: DVE 2x tensor_scalar multiply + adds split by free-dim
            # halves so the first eout DMA starts while the second half adds
            f = fts[(0, K - 1)][:, 0, :]
            nc.vector.tensor_scalar(
                out=f, in0=f, scalar1=w[:, K - 1:K], scalar2=None,
                op0=mybir.AluOpType.mult)
            for hh in range(2):
                sl = slice(hh * (LD // 2), (hh + 1) * (LD // 2))
                nc.vector.tensor_tensor(out=acc[:, sl], in0=acc[:, sl],
                                        in1=f[:, sl], op=mybir.AluOpType.add)
                nc.sync.dma_start(eout[0:128, sl], acc[:, sl])

            # --- chunk 1 weighted sum: diag matmuls accumulating in PSUM ---
            psum = pspool.tile([128, LD], F32)
            for k in range(K):
                for s in range(LD // MM_N):
                    nc.tensor.matmul(
                        out=psum[:, s * MM_N:(s + 1) * MM_N],
                        lhsT=wds[k][:],
                        rhs=fts[(1, k)][:, 0, s * MM_N:(s + 1) * MM_N],
                        start=(k == 0), stop=(k == K - 1))
            for h in range(2):
                o = apool.tile([128, LD // 2], F32, tag="o")
                nc.scalar.copy(o[:], psum[:, h * (LD // 2):(h + 1) * (LD // 2)])
                nc.sync.dma_start(
                    eout[128:256, h * (LD // 2):(h + 1) * (LD // 2)], o[:])

    nc.compile()
    return nc


def prep_inputs(similarity_matrix, batch_emb_om, index):
    """Shard + marshal full inputs into per-core input maps."""
    sim = np.ascontiguousarray(similarity_matrix, dtype=np.float32)
    emb = np.ascontiguousarray(
        np.asarray(batch_emb_om, dtype=np.float32).reshape(N, LD))
    idx = np.asarray(index, dtype=np.int64).astype(np.int32)

    iotac = np.tile(np.arange(SBLK, dtype=np.float32), (128, 1))
    identc = np.eye(128, dtype=np.float32)

    def make_cst(rmod_d):
        return np.concatenate([rmod_d, iotac, identc], axis=1)

    def wrap(lst):
        # dma_gather order: item i read from idxs[i%16, i//16]; replicate to
        # all 8 GPSIMD core groups (16 partitions each).
        return np.tile(lst.astype(np.int16).reshape(-1, 16).T, (8, 1))

    in_maps = []
    for d in range(NCORES):
        idx_d = idx[d * RPD:(d + 1) * RPD]            # [256, K]
        gidx_d = np.zeros((128, K * 16 + NCH * K * 8), dtype=np.int16)
        rmod_d = np.zeros((128, NCH * K), dtype=np.float32)
        rows_all = np.arange(RPD)
        for k in range(K):
            cols = idx_d[rows_all, k]
            sim_lst = rows_all * NBLK + (cols >> 6)        # [256] both chunks
            gidx_d[:, k * 16:(k + 1) * 16] = wrap(sim_lst)
        for c in range(NCH):
            rows = np.arange(c * 128, (c + 1) * 128)
            for k in range(K):
                cols = idx_d[rows, k]
                ci = K * 16 + (c * K + k) * 8
                gidx_d[:, ci:ci + 8] = wrap(cols)
                rmod_d[:, c * K + k] = (cols & 63).astype(np.float32)
        in_maps.append({
            "emb": emb,
            "simb": sim[d * RPD:(d + 1) * RPD].reshape(RPD * NBLK, SBLK),
            "gidx": gidx_d,
            "cst": make_cst(rmod_d),
        })
    return in_maps


def assemble_outputs(results):
    wfull = np.concatenate([r["wout"] for r in results], axis=0)
    efull = np.concatenate([r["eout"] for r in results], axis=0)
    return wfull.reshape(N, K), efull.reshape(N, L, D)


def kernel(similarity_matrix, batch_emb_om, index, trace=False):
    if trace:
        _install_axon_ntff_hook()
    if "nc" not in _cache:
        _cache["nc"] = build()
    nc = _cache["nc"]
    in_maps = prep_inputs(similarity_matrix, batch_emb_om, index)
    res = run_bass_kernel_spmd(nc, in_maps, core_ids=list(range(NCORES)),
                               trace=trace)
    out = assemble_outputs(res.results)
    if trace:
        _cache["last_result"] = res
    return out
